# revision 1
# baseline (speedup 1.0000x reference)
"""v3: grid + cubic-interpolation kernel for ChannelwiseSpatialMHSA.

Instead of exp-ing all 4 heads x 1024 s-rows per sequence (32 [128,1024]
activation tiles), evaluate the softmax-weighted mean w(a) = sum_t
softmax_t(a*x_t)*x_t on a G=512 uniform grid of tilts a (4 tiles), then
cubic-interpolate at the 4096 query tilts a = c_h*x_s via a GpSimd ap_gather
of per-segment Catmull-Rom coefficients. Max |w| error ~6e-5 (measured
offline), output scale-relative error ~1e-5.

Layout notes:
- grid tile b: partitions = grid rows g = b*128+p, free = t (1024)
- ap_gather groups (16 partitions each) = (half, h): g = half*4 + h; group g
  gathers its 512 queries (s in [512*half, 512*half+512)), j = s-within-half
- idx for query j of group g lives at partition 16g + j%16, col j//16
"""

import numpy as np

B, HH, WW, C = 2, 32, 32, 32
S = 1024
D = 64
NH = 4
DH = 16
NCORES = 8
NSEQ = 8
G = 512
NGB = G // 128  # grid blocks = 4
WGPAD = 520  # padded wg row length in dram

_CACHE = {}


def _build_nc():
    import concourse.bacc as bacc
    import concourse.bass as bass
    import concourse.tile as tile
    from concourse import mybir, library_config

    f32 = mybir.dt.float32
    i16 = mybir.dt.int16
    Alu = mybir.AluOpType
    Act = mybir.ActivationFunctionType

    nc = bacc.Bacc()

    xs = nc.dram_tensor("xs", [NSEQ, S], f32, kind="ExternalInput")
    xe = nc.dram_tensor("xe", [NSEQ, 5], f32, kind="ExternalInput")  # -max,-min,amax,k1,merge
    embed_w = nc.dram_tensor("embed_w", [D, 1], f32, kind="ExternalInput")
    q_w = nc.dram_tensor("q_w", [D, D], f32, kind="ExternalInput")
    k_w = nc.dram_tensor("k_w", [D, D], f32, kind="ExternalInput")
    v_w = nc.dram_tensor("v_w", [D, D], f32, kind="ExternalInput")
    o_w = nc.dram_tensor("o_w", [D, D], f32, kind="ExternalInput")
    hmask = nc.dram_tensor("hmask", [D, NH], f32, kind="ExternalInput")
    ident = nc.dram_tensor("ident", [128, 128], f32, kind="ExternalInput")
    ucol = nc.dram_tensor("ucol", [128, NGB], f32, kind="ExternalInput")
    gidx = nc.dram_tensor("gidx", [128, NGB], f32, kind="ExternalInput")
    outp = nc.dram_tensor("outp", [S, D], f32, kind="ExternalOutput")

    c_dram = nc.dram_tensor("c_scratch", [1, NH], f32)
    wg_drams = [
        nc.dram_tensor(f"wg_scratch{i}", [1, WGPAD], f32) for i in range(NSEQ)
    ]
    ctab_drams = [
        nc.dram_tensor(f"ctab_scratch{i}", [1, G * 5], f32) for i in range(NSEQ)
    ]

    def rawap(handle, offset, ap):
        base = handle[:, :]
        return bass.AP(tensor=base.tensor, offset=offset, ap=ap)

    with tile.TileContext(nc) as tc:
        with (
            tc.tile_pool(name="consts", bufs=1) as consts,
            tc.tile_pool(name="seq", bufs=3) as seqp,
            tc.tile_pool(name="rows", bufs=2) as rowsp,
            tc.tile_pool(name="big", bufs=2) as bigp,
            tc.tile_pool(name="lhsp", bufs=4) as lhsp,
            tc.tile_pool(name="et", bufs=6) as etp,
            tc.tile_pool(name="scr", bufs=3) as scrp,
            tc.tile_pool(name="small", bufs=12) as smallp,
            tc.tile_pool(name="ps", bufs=2, space="PSUM") as psp,
            tc.tile_pool(name="ps1", bufs=1, space="PSUM") as psp1,
            tc.tile_pool(name="xps", bufs=1, space="PSUM") as xpsp,
            tc.tile_pool(name="t4ps", bufs=2, space="PSUM") as t4psp,
            tc.tile_pool(name="mmps", bufs=1, space="PSUM") as mmps,
        ):
            nc.gpsimd.load_library(library_config.ap_gather)

            # ---- prologue: fold weights into c[1,4] and U[4,64] ----
            ew_sb = consts.tile([D, 1], f32)
            nc.sync.dma_start(out=ew_sb, in_=embed_w[:, :])
            qT_sb = consts.tile([D, D], f32)
            nc.sync.dma_start(out=qT_sb, in_=q_w.rearrange("o i -> i o"))
            kT_sb = consts.tile([D, D], f32)
            nc.sync.dma_start(out=kT_sb, in_=k_w.rearrange("o i -> i o"))
            vT_sb = consts.tile([D, D], f32)
            nc.sync.dma_start(out=vT_sb, in_=v_w.rearrange("o i -> i o"))
            oT_sb = consts.tile([D, D], f32)
            nc.sync.dma_start(out=oT_sb, in_=o_w.rearrange("o d -> d o"))
            hm_sb = consts.tile([D, NH], f32)
            nc.sync.dma_start(out=hm_sb, in_=hmask[:, :])
            id_sb = consts.tile([128, 128], f32)
            nc.sync.dma_start(out=id_sb, in_=ident[:, :])
            u_col = consts.tile([128, NGB], f32)
            nc.sync.dma_start(out=u_col, in_=ucol[:, :])
            gi_sb = consts.tile([128, NGB], f32)
            nc.sync.dma_start(out=gi_sb, in_=gidx[:, :])
            ones_sb = consts.tile([1, 128], f32)
            nc.vector.memset(ones_sb, 1.0)

            vec_sb = {}
            for name, wT in (("q", qT_sb), ("k", kT_sb), ("v", vT_sb)):
                vps = psp1.tile([D, 1], f32, tag="pro")
                nc.tensor.matmul(vps, lhsT=wT, rhs=ew_sb, start=True, stop=True)
                vsb = consts.tile([D, 1], f32, tag=f"{name}vec")
                nc.vector.tensor_copy(vsb, vps)
                vec_sb[name] = vsb

            kvs_sb = consts.tile([D, 1], f32)
            nc.vector.tensor_scalar_mul(kvs_sb, vec_sb["k"], 1.0 / np.sqrt(DH))
            mq_sb = consts.tile([D, NH], f32)
            nc.vector.tensor_scalar_mul(mq_sb, hm_sb, vec_sb["q"])
            mv_sb = consts.tile([D, NH], f32)
            nc.vector.tensor_scalar_mul(mv_sb, hm_sb, vec_sb["v"])

            c_ps = psp1.tile([1, NH], f32, tag="pro")
            nc.tensor.matmul(c_ps, lhsT=kvs_sb, rhs=mq_sb, start=True, stop=True)
            c_sb = consts.tile([1, NH], f32)
            nc.vector.tensor_copy(c_sb, c_ps)
            nc.sync.dma_start(out=c_dram[:, :], in_=c_sb)
            # c_ghost[p] = c[(p//16)%4]  (group layout (half, h, r))
            c_ghost = consts.tile([128, 1], f32)
            for g in range(8):
                nc.sync.dma_start(
                    out=c_ghost[16 * g : 16 * g + 16, :],
                    in_=c_dram[0:1, g % 4 : g % 4 + 1].to_broadcast([16, 1]),
                )

            u_ps = psp1.tile([NH, D], f32, tag="pro")
            nc.tensor.matmul(u_ps, lhsT=mv_sb, rhs=oT_sb, start=True, stop=True)
            u_sb = consts.tile([NH, D], f32)
            nc.vector.tensor_copy(u_sb, u_ps)

            NSB = 8
            acc_ps = mmps.tile([128, NSB, D], f32, tag="accps")

            def grid_phase(n):
                xrow = rowsp.tile([1, S], f32, tag="xrow")
                nc.sync.dma_start(out=xrow, in_=xs[n : n + 1, :])
                x_ps = xpsp.tile([128, S], f32, tag="xps")
                for hf in range(2):
                    nc.tensor.matmul(
                        x_ps[:, 512 * hf : 512 * (hf + 1)],
                        lhsT=ones_sb,
                        rhs=xrow[:, 512 * hf : 512 * (hf + 1)],
                        start=True,
                        stop=True,
                    )
                x_bc = seqp.tile([128, S], f32, tag="xbc")
                nc.scalar.copy(x_bc, x_ps)
                xem = seqp.tile([128, 5], f32, tag="xem")
                nc.sync.dma_start(out=xem, in_=xe[n : n + 1, :].to_broadcast([128, 5]))
                nxmax = xem[:, 0:1]
                nxmin = xem[:, 1:2]
                amax_bc = xem[:, 2:3]
                k1_bc = xem[:, 3:4]
                ck1 = seqp.tile([128, 1], f32, tag="ck1")
                nc.vector.tensor_mul(ck1, c_ghost, k1_bc)
                mu_sb = seqp.tile([NH, D], f32, tag="mu")
                nc.vector.tensor_scalar_mul(mu_sb, u_sb, xem[0:NH, 4:5])

                scale_m = smallp.tile([128, NGB], f32, tag="scale")
                nc.vector.tensor_scalar_mul(scale_m, u_col, amax_bc)
                t1 = smallp.tile([128, NGB], f32, tag="t1")
                nc.vector.tensor_scalar_mul(t1, scale_m, nxmax)
                t2 = smallp.tile([128, NGB], f32, tag="t2")
                nc.vector.tensor_scalar_mul(t2, scale_m, nxmin)
                bias_m = smallp.tile([128, NGB], f32, tag="bias")
                nc.vector.tensor_tensor(bias_m, t1, t2, op=Alu.min)

                wg_all = seqp.tile([128, NGB], f32, tag="wgall")
                for b in range(NGB):
                    den = smallp.tile([128, 1], f32, tag="den")
                    et = etp.tile([128, S], f32, tag="et")
                    nc.scalar.activation(
                        out=et,
                        in_=x_bc,
                        func=Act.Exp,
                        scale=scale_m[:, b : b + 1],
                        bias=bias_m[:, b : b + 1],
                        accum_out=den,
                    )
                    rec = smallp.tile([128, 1], f32, tag="rec")
                    nc.vector.reciprocal(rec, den)
                    scr = scrp.tile([128, S], f32, tag="scr")
                    nc.vector.scalar_tensor_tensor(
                        out=scr,
                        in0=et,
                        scalar=rec,
                        in1=x_bc,
                        op0=Alu.mult,
                        op1=Alu.mult,
                        accum_out=wg_all[:, b : b + 1],
                    )

                wgt_ps = psp.tile([NGB, 128], f32, tag="wgT")
                nc.tensor.transpose(wgt_ps, wg_all[:, :], id_sb)
                wgt_sb = seqp.tile([NGB, 128], f32, tag="wgTsb")
                nc.vector.tensor_copy(wgt_sb, wgt_ps)
                nc.sync.dma_start(
                    out=rawap(wg_drams[n], 1, [[128, NGB], [1, 128]]),
                    in_=wgt_sb,
                )
                nc.sync.dma_start(
                    out=wg_drams[n][0:1, G + 1 : G + 4],
                    in_=wgt_sb[NGB - 1 : NGB, 125:128],
                )
                wsh4 = smallp.tile([128, NGB, 4], f32, tag="wsh4")
                nc.sync.dma_start(
                    out=wsh4,
                    in_=rawap(
                        wg_drams[n], 1, [[1, 128], [128, NGB], [1, 4]]
                    ),
                )
                p0 = wsh4[:, :, 0]
                p1 = wsh4[:, :, 1]
                p2 = wsh4[:, :, 2]
                p3 = wsh4[:, :, 3]
                Ct = seqp.tile([128, NGB, 5], f32, tag="C")
                nc.vector.tensor_copy(Ct[:, :, 0], p1)
                tt = smallp.tile([128, NGB], f32, tag="ct1")
                nc.vector.tensor_tensor(tt, p2, p0, op=Alu.subtract)
                nc.vector.tensor_scalar_mul(Ct[:, :, 1], tt, 0.5)
                u1 = smallp.tile([128, NGB], f32, tag="ct2")
                nc.vector.tensor_tensor(u1, p3, p0, op=Alu.subtract)
                u2 = smallp.tile([128, NGB], f32, tag="ct3")
                nc.vector.tensor_tensor(u2, p1, p2, op=Alu.subtract)
                t3 = smallp.tile([128, NGB], f32, tag="ct4")
                nc.vector.scalar_tensor_tensor(
                    out=t3, in0=u2, scalar=3.0, in1=u1, op0=Alu.mult, op1=Alu.add
                )
                nc.vector.tensor_scalar_mul(Ct[:, :, 3], t3, 0.5)
                t4 = smallp.tile([128, NGB], f32, tag="ct5")
                nc.vector.tensor_tensor(t4, p2, p1, op=Alu.subtract)
                t5 = smallp.tile([128, NGB], f32, tag="ct6")
                nc.vector.tensor_tensor(t5, t4, Ct[:, :, 1], op=Alu.subtract)
                nc.vector.tensor_tensor(Ct[:, :, 2], t5, Ct[:, :, 3], op=Alu.subtract)
                nc.vector.tensor_copy(Ct[:, :, 4], gi_sb)
                nc.sync.dma_start(
                    out=rawap(
                        ctab_drams[n], 0, [[5, 128], [128 * 5, NGB], [1, 5]]
                    ),
                    in_=Ct,
                )
                crow = rowsp.tile([1, G * 5], f32, tag="crow")
                nc.sync.dma_start(out=crow, in_=ctab_drams[n][0:1, :])
                t4_sb = bigp.tile([128, G * 5], f32, tag="T4")
                for ci in range(5):
                    t4_ps = t4psp.tile([128, 512], f32, tag="t4ps")
                    nc.tensor.matmul(
                        t4_ps,
                        lhsT=ones_sb,
                        rhs=crow[:, 512 * ci : 512 * (ci + 1)],
                        start=True,
                        stop=True,
                    )
                    nc.scalar.copy(t4_sb[:, 512 * ci : 512 * (ci + 1)], t4_ps)

                xg = smallp.tile([128, 32], f32, tag="xg")
                for g in range(8):
                    half = g // 4
                    nc.sync.dma_start(
                        out=xg[16 * g : 16 * g + 16, :],
                        in_=rawap(xs, n * S + 512 * half, [[1, 16], [16, 32]]),
                    )
                v32 = smallp.tile([128, 32], f32, tag="v32")
                nc.vector.tensor_scalar(
                    out=v32,
                    in0=xg,
                    scalar1=ck1,
                    scalar2=(G - 1) / 2.0 - 1.5,
                    op0=Alu.mult,
                    op1=Alu.add,
                )
                v32c = smallp.tile([128, 32], f32, tag="v32c")
                nc.vector.tensor_scalar(
                    out=v32c,
                    in0=v32,
                    scalar1=1.0,
                    scalar2=float(G - 5),
                    op0=Alu.max,
                    op1=Alu.min,
                )
                idx32 = smallp.tile([128, 32], i16, tag="idx32")
                nc.vector.tensor_copy(idx32, v32c)

                y2 = seqp.tile([128, 512], f32, tag="y2")
                for half in range(2):
                    lo = 64 * half
                    nc.vector.tensor_scalar(
                        out=y2[lo : lo + 64, :],
                        in0=x_bc[lo : lo + 64, 512 * half : 512 * half + 512],
                        scalar1=ck1[lo : lo + 64, :],
                        scalar2=(G - 1) / 2.0,
                        op0=Alu.mult,
                        op1=Alu.add,
                    )
                return dict(n=n, t4_sb=t4_sb, idx32=idx32, y2=y2, mu_sb=mu_sb)

            def interp_phase(st):
                n = st["n"]
                gq = bigp.tile([128, 512, 5], f32, tag="gq")
                nc.gpsimd.ap_gather(
                    out_ap=gq,
                    in_ap=st["t4_sb"],
                    idxs_ap=st["idx32"],
                    channels=128,
                    num_elems=G,
                    d=5,
                    num_idxs=512,
                )
                f_t = seqp.tile([128, 512], f32, tag="ft")
                nc.vector.tensor_tensor(f_t, st["y2"], gq[:, :, 4], op=Alu.subtract)
                hh = seqp.tile([128, 512], f32, tag="hh")
                nc.vector.tensor_tensor(hh, gq[:, :, 3], f_t, op=Alu.mult)
                nc.vector.tensor_tensor(hh, hh, gq[:, :, 2], op=Alu.add)
                nc.vector.tensor_tensor(hh, hh, f_t, op=Alu.mult)
                nc.vector.tensor_tensor(hh, hh, gq[:, :, 1], op=Alu.add)
                nc.vector.tensor_tensor(hh, hh, f_t, op=Alu.mult)
                w_q = seqp.tile([128, 512], f32, tag="wq")
                nc.vector.tensor_tensor(w_q, hh, gq[:, :, 0], op=Alu.add)

                for half in range(2):
                    lhsT = lhsp.tile([NH, 512], f32, tag="lhsT")
                    lo = 64 * half
                    nc.sync.dma_start(out=lhsT, in_=w_q[lo : lo + 64 : 16, :])
                    for chunk in range(4):
                        sb = 4 * half + chunk
                        nc.tensor.matmul(
                            acc_ps[:, sb, :],
                            lhsT=lhsT[:, 128 * chunk : 128 * (chunk + 1)],
                            rhs=st["mu_sb"],
                            start=(n == 0 and half == 0 and chunk == 0),
                            stop=(n == NSEQ - 1 and half == 1 and chunk == 3),
                            skip_group_check=True,
                        )

            prev = None
            for n in range(NSEQ):
                st = grid_phase(n)
                if prev is not None:
                    interp_phase(prev)
                prev = st
            interp_phase(prev)

            out_sb = consts.tile([128, NSB, D], f32)
            nc.vector.tensor_copy(out_sb, acc_ps)
            nc.sync.dma_start(
                out=outp.rearrange("(sb p) o -> p sb o", p=128), in_=out_sb
            )

    if not nc.is_finalized():
        nc.finalize()
    return nc


def _host_inputs(x, embed_w, q_w, k_w, v_w, o_w, merge_w):
    t = np.ascontiguousarray(
        np.asarray(x, np.float32).transpose(0, 3, 1, 2).reshape(B * C, S)
    )
    hmask = np.repeat(np.eye(NH, dtype=np.float32), DH, axis=0)
    ident = np.eye(128, dtype=np.float32)
    g = np.arange(128)[:, None] + 128 * np.arange(NGB)[None, :]
    ucol = (-1.0 + 2.0 * g / (G - 1)).astype(np.float32)
    gidx = (g + 1).astype(np.float32)
    # host-side grid-placement constants: amax bounds the query tilts
    # c_h * x_s; the device uses them only to place the interpolation grid
    ew = np.asarray(embed_w, np.float64)[:, 0]
    qv = np.asarray(q_w, np.float64) @ ew
    kv = np.asarray(k_w, np.float64) @ ew
    cmax = max(
        abs(qv[DH * h : DH * (h + 1)] @ kv[DH * h : DH * (h + 1)]) / np.sqrt(DH)
        for h in range(NH)
    )
    in_maps = []
    for k in range(NCORES):
        sl = np.ascontiguousarray(t[NSEQ * k : NSEQ * (k + 1)])
        amax = (cmax * np.abs(sl).max(axis=1)).astype(np.float32)
        k1 = ((G - 1) / 2.0 / amax.astype(np.float64)).astype(np.float32)
        chans = np.arange(NSEQ * k, NSEQ * (k + 1)) % C
        mslice = np.asarray(merge_w, np.float32)[0, chans]
        xe = np.stack(
            [-sl.max(axis=1), -sl.min(axis=1), amax, k1, mslice], axis=1
        ).astype(np.float32)
        in_maps.append(
            dict(
                xs=sl,
                xe=np.ascontiguousarray(xe),
                embed_w=np.asarray(embed_w, np.float32),
                q_w=np.asarray(q_w, np.float32),
                k_w=np.asarray(k_w, np.float32),
                v_w=np.asarray(v_w, np.float32),
                o_w=np.asarray(o_w, np.float32),
                hmask=hmask,
                ident=ident,
                ucol=np.ascontiguousarray(ucol),
                gidx=np.ascontiguousarray(gidx),
            )
        )
    return in_maps


def kernel(x, embed_w, q_w, k_w, v_w, o_w, merge_w):
    from concourse.bass_utils import run_bass_kernel_spmd

    if "nc" not in _CACHE:
        _CACHE["nc"] = _build_nc()
    nc = _CACHE["nc"]
    in_maps = _host_inputs(x, embed_w, q_w, k_w, v_w, o_w, merge_w)
    res = run_bass_kernel_spmd(nc, in_maps, core_ids=list(range(NCORES)))
    out = np.zeros((B, S, D), dtype=np.float32)
    for k in range(NCORES):
        out[k // (NCORES // B)] += res.results[k]["outp"]
    return out.reshape(B, HH, WW, D)



# revision 14
# speedup vs baseline: 1.4077x; 1.4077x over previous
"""v4: grid + linear-interpolation kernel for ChannelwiseSpatialMHSA.

Scores are rank-1: attn(s,t) = softmax_t(c_h*x_s*x_t), so the attention
output per (seq, head, s) is w(c_h*x_s)*u_h where w(a) = sum_t
softmax_t(a*x_t)*x_t = d/da ln Z(a), Z(a) = sum_t e^{a*x_t}.

Per sequence we evaluate ln Z on a G=128 uniform tilt grid (one exp
activation with per-partition scale/bias and accum), differentiate it
with a 6th-order central difference to get w on the grid, and linearly
interpolate at the 4096 query tilts via a GpSimd ap_gather of
[w, dw, gpos] triples. Offline-validated rel err ~9e-4 (vs 2e-2 tol).

Layout notes:
- grid: partition g = grid row; exp runs on [128, S] broadcast of x.
- gather groups (16 partitions) = (half, h): g = half*4 + h; group g
  gathers its 512 queries (s in [512*half, 512*half+512)).
- idx for query j of group g lives at partition 16g + j%16, col j//16.
- final: out[s,:] = sum_{n,h} wq[(n,h), s] * mu[(n,h), :] as 8 matmuls
  contracting all 32 (seq, head) pairs at once.
"""

import numpy as np

B, HH, WW, C = 2, 32, 32, 32
S = 1024
D = 64
NH = 4
DH = 16
NCORES = 8
NSEQ = 8
G = 128
CENTER = (G - 1) / 2.0  # 63.5
SPAN = 119.0  # queries map to v in [4, 123]
VLO, VHI = 3.0, 123.0  # idx clamp (entries 3..123 valid for linear)

_CACHE = {}


def _build_nc():
    import concourse.bacc as bacc
    import concourse.bass as bass
    import concourse.tile as tile
    from concourse import mybir, library_config

    f32 = mybir.dt.float32
    i16 = mybir.dt.int16
    Alu = mybir.AluOpType
    Act = mybir.ActivationFunctionType

    nc = bacc.Bacc()

    xs = nc.dram_tensor("xs", [NSEQ, S], f32, kind="ExternalInput")
    gsb = nc.dram_tensor("gsb", [128, 2 * NSEQ], f32, kind="ExternalInput")
    biasr = nc.dram_tensor("biasr", [NSEQ, G], f32, kind="ExternalInput")
    scqexp = nc.dram_tensor("scqexp", [128, 32 * NSEQ], f32, kind="ExternalInput")
    k160 = nc.dram_tensor("k160", [NSEQ, 1], f32, kind="ExternalInput")
    mu = nc.dram_tensor("mu", [NH * NSEQ, D], f32, kind="ExternalInput")
    gposr = nc.dram_tensor("gposr", [1, G], f32, kind="ExternalInput")
    ident = nc.dram_tensor("ident", [128, 128], f32, kind="ExternalInput")
    outp = nc.dram_tensor("outp", [S, D], f32, kind="ExternalOutput")
    ctab_dram = nc.dram_tensor("ctab_scratch", [NSEQ, 3 * G], f32)

    def rawap(handle, offset, ap):
        base = handle[:, :]
        return bass.AP(tensor=base.tensor, offset=offset, ap=ap)

    with tile.TileContext(nc) as tc:
        with (
            tc.tile_pool(name="consts", bufs=1) as consts,
            tc.tile_pool(name="et", bufs=2) as etp,
            tc.tile_pool(name="gq", bufs=2) as gqp,
            tc.tile_pool(name="fw", bufs=2) as fwp,
            tc.tile_pool(name="ps", bufs=1, space="PSUM") as psp,
            tc.tile_pool(name="accps", bufs=1, space="PSUM") as accp,
        ):
            nc.gpsimd.load_library(library_config.ap_gather)

            gsb_sb = consts.tile([128, 2 * NSEQ], f32)
            nc.sync.dma_start(out=gsb_sb, in_=gsb[:, :])
            scq_sb = consts.tile([128, NSEQ, 32], f32)
            nc.sync.dma_start(out=scq_sb, in_=scqexp[:, :])
            mu_sb = consts.tile([NH * NSEQ, D], f32)
            nc.sync.dma_start(out=mu_sb, in_=mu[:, :])
            id_sb = consts.tile([128, 128], f32)
            nc.sync.dma_start(out=id_sb, in_=ident[:, :])
            k160_sb = consts.tile([NSEQ, 1], f32)
            nc.sync.dma_start(out=k160_sb, in_=k160[:, :])
            biasr_sb = consts.tile([NSEQ, G], f32)
            nc.sync.dma_start(out=biasr_sb, in_=biasr[:, :])

            ctabs = consts.tile([NSEQ, G, 3], f32)
            nc.sync.dma_start(
                out=ctabs[:, :, 2], in_=gposr[0:1, :].to_broadcast([NSEQ, G])
            )

            # queries in gather wrap layout: gather position i = 16*mm + rr
            # of group g=(half,h) is query s = 512*half + 32*rr + mm, so
            # xg_all[16g+rr, n, mm] = x[n, 512*(g//4) + 32*rr + mm]
            xg_all = consts.tile([128, NSEQ, 32], f32)
            for grp in range(8):
                half = grp // 4
                nc.scalar.dma_start(
                    out=xg_all[16 * grp : 16 * grp + 16, :, :],
                    in_=rawap(xs, 512 * half, [[32, 16], [1024, NSEQ], [1, 32]]),
                )
            v1 = consts.tile([128, NSEQ, 32], f32)
            nc.vector.tensor_tensor(v1, xg_all, scq_sb, op=Alu.mult)
            v2 = consts.tile([128, NSEQ, 32], f32)
            nc.vector.tensor_scalar(
                out=v2, in0=v1, scalar1=CENTER - 0.5, scalar2=VLO,
                op0=Alu.add, op1=Alu.max,
            )
            idx_all = consts.tile([128, NSEQ, 32], i16)
            nc.vector.tensor_scalar(
                out=idx_all, in0=v2, scalar1=VHI, scalar2=None, op0=Alu.min
            )

            # ---- phase A: exp on the tilt grid, accumulate den ----
            dwg = consts.tile([128, NSEQ], f32)
            xbc = []
            for pair in range(NSEQ // 2):
                xb = consts.tile([128, 2, 2, 16, 32], f32, tag=f"xbc{pair}")
                nc.sync.dma_start(
                    out=xb, in_=rawap(xs, pair * 2 * S, [[0, 128], [1, 2 * S]])
                )
                xbc.append(xb)
            for n in range(NSEQ):
                et = etp.tile([128, 2, 16, 32], f32, tag="et")
                nc.scalar.activation(
                    out=et,
                    in_=xbc[n // 2][:, n % 2, :, :, :],
                    func=Act.Exp,
                    scale=gsb_sb[:, 2 * n : 2 * n + 1],
                    bias=gsb_sb[:, 2 * n + 1 : 2 * n + 2],
                    accum_out=dwg[:, n : n + 1],
                )

            # ---- phase B: lnZ rows, d6 derivative, linear table ----
            tp_ps = psp.tile([NSEQ, 128], f32, tag="tp")
            nc.tensor.transpose(tp_ps, dwg, id_sb)
            lnrow = consts.tile([NSEQ, G], f32)
            nc.scalar.activation(out=lnrow, in_=tp_ps, func=Act.Ln)
            lc = consts.tile([NSEQ, G], f32)
            nc.vector.tensor_tensor(lc, lnrow, biasr_sb, op=Alu.subtract)
            t1 = consts.tile([NSEQ, G - 6], f32)
            nc.vector.tensor_tensor(t1, lc[:, 4:126], lc[:, 2:124], op=Alu.subtract)
            t2 = consts.tile([NSEQ, G - 6], f32)
            nc.vector.tensor_tensor(t2, lc[:, 5:127], lc[:, 1:123], op=Alu.subtract)
            t3 = consts.tile([NSEQ, G - 6], f32)
            nc.vector.tensor_tensor(t3, lc[:, 6:128], lc[:, 0:122], op=Alu.subtract)
            u1 = consts.tile([NSEQ, G - 6], f32)
            nc.vector.scalar_tensor_tensor(
                out=u1, in0=t2, scalar=-9.0, in1=t3, op0=Alu.mult, op1=Alu.add
            )
            wfull = consts.tile([NSEQ, G], f32)
            nc.vector.scalar_tensor_tensor(
                out=wfull[:, 3:125], in0=t1, scalar=45.0, in1=u1,
                op0=Alu.mult, op1=Alu.add,
            )
            nc.vector.tensor_scalar(
                out=wfull[:, 3:125], in0=wfull[:, 3:125], scalar1=k160_sb,
                scalar2=None, op0=Alu.mult,
            )
            # table entries e in [3,123]: c0 = w[e], c1 = w[e+1]-w[e]
            nc.vector.tensor_copy(ctabs[:, 3:124, 0], wfull[:, 3:124])
            nc.vector.tensor_tensor(
                ctabs[:, 3:124, 1], wfull[:, 4:125], wfull[:, 3:124], op=Alu.subtract
            )
            nc.sync.dma_start(out=ctab_dram[:, :], in_=ctabs)
            tab_all = consts.tile([128, NSEQ * 3 * G], f32)
            nc.sync.dma_start(
                out=tab_all, in_=rawap(ctab_dram, 0, [[0, 128], [1, NSEQ * 3 * G]])
            )

            # ---- phase G: gather + linear interp + wq rows ----
            wq_sb = consts.tile([NH * NSEQ, S], f32)
            for n in range(NSEQ):
                gq = gqp.tile([128, 32, 16, 3], f32, tag="gq")
                nc.gpsimd.ap_gather(
                    out_ap=gq,
                    in_ap=tab_all[:, 3 * G * n : 3 * G * (n + 1)],
                    idxs_ap=idx_all[:, n, :],
                    channels=128,
                    num_elems=G,
                    d=3,
                    num_idxs=512,
                )
                f_t = fwp.tile([128, 32, 16], f32, tag="ft")
                for half in range(2):
                    lo = 64 * half
                    nc.vector.scalar_tensor_tensor(
                        out=f_t[lo : lo + 64, :, :],
                        in0=xbc[n // 2][lo : lo + 64, n % 2, half, :, :].transpose(
                            [0, 2, 1]
                        ),
                        scalar=scq_sb[lo : lo + 64, n, 0:1],
                        in1=gq[lo : lo + 64, :, :, 2],
                        op0=Alu.mult,
                        op1=Alu.subtract,
                    )
                wq = fwp.tile([128, 32, 16], f32, tag="wq")
                nc.vector.tensor_tensor(wq, f_t, gq[:, :, :, 1], op=Alu.mult)
                nc.vector.tensor_tensor(wq, wq, gq[:, :, :, 0], op=Alu.add)
                for half in range(2):
                    nc.scalar.dma_start(
                        out=wq_sb[NH * n : NH * (n + 1), 512 * half : 512 * (half + 1)],
                        in_=wq[64 * half : 64 * half + 64 : 16, :, :],
                    )

            # ---- final: out[s,:] = sum_{n,h} wq[(n,h),s] * mu[(n,h),:] ----
            acc_ps = accp.tile([128, NSEQ, D], f32, tag="acc")
            for c in range(8):
                nc.tensor.matmul(
                    acc_ps[:, c, :],
                    lhsT=wq_sb[:, 128 * c : 128 * (c + 1)],
                    rhs=mu_sb,
                    start=True,
                    stop=True,
                )
            out_sb = consts.tile([128, NSEQ, D], f32)
            nc.scalar.copy(out_sb, acc_ps)
            nc.sync.dma_start(
                out=outp.rearrange("(sb p) o -> p sb o", p=128), in_=out_sb
            )

    if not nc.is_finalized():
        nc.finalize()
    return nc


def _host_inputs(x, embed_w, q_w, k_w, v_w, o_w, merge_w):
    t = np.ascontiguousarray(
        np.asarray(x, np.float32).transpose(0, 3, 1, 2).reshape(B * C, S)
    )
    ew = np.asarray(embed_w, np.float64)[:, 0]
    qv = np.asarray(q_w, np.float64) @ ew
    kv = np.asarray(k_w, np.float64) @ ew
    vv = np.asarray(v_w, np.float64) @ ew
    o64 = np.asarray(o_w, np.float64)
    c_h = np.array(
        [qv[DH * h : DH * (h + 1)] @ kv[DH * h : DH * (h + 1)] for h in range(NH)]
    ) / np.sqrt(DH)
    u = np.stack(
        [o64[:, DH * h : DH * (h + 1)] @ vv[DH * h : DH * (h + 1)] for h in range(NH)]
    )  # [NH, D]
    cmax = np.abs(c_h).max()
    merge = np.asarray(merge_w, np.float64)[0]
    ident = np.eye(128, dtype=np.float32)
    gpos = (np.arange(G, dtype=np.float64) - CENTER).astype(np.float32).reshape(1, G)
    gg = np.arange(G, dtype=np.float64)

    in_maps = []
    for k in range(NCORES):
        sl = np.ascontiguousarray(t[NSEQ * k : NSEQ * (k + 1)])
        gsb = np.zeros((128, 2 * NSEQ), np.float32)
        biasr = np.zeros((NSEQ, G), np.float32)
        scqexp = np.zeros((128, 32 * NSEQ), np.float32)
        k160 = np.zeros((NSEQ, 1), np.float32)
        mu = np.zeros((NH * NSEQ, D), np.float32)
        for n in range(NSEQ):
            xr = sl[n].astype(np.float64)
            amax = cmax * np.abs(xr).max()
            k1 = SPAN / (2 * amax)
            a_g = (gg - CENTER) / k1
            b_g = -np.maximum(a_g * xr.max(), a_g * xr.min())
            gsb[:, 2 * n] = a_g
            gsb[:, 2 * n + 1] = b_g
            biasr[n] = b_g
            k160[n, 0] = k1 / 60.0
            p = np.arange(128)
            scqexp[:, 32 * n : 32 * (n + 1)] = (
                c_h[(p // 16) % 4] * k1
            )[:, None]
            chan = (NSEQ * k + n) % C
            mu[NH * n : NH * (n + 1), :] = (merge[chan] * u).astype(np.float32)
        in_maps.append(
            dict(
                xs=sl,
                gsb=gsb,
                biasr=biasr,
                scqexp=scqexp,
                k160=k160,
                mu=mu,
                gposr=gpos,
                ident=ident,
            )
        )
    return in_maps


def kernel(x, embed_w, q_w, k_w, v_w, o_w, merge_w):
    from concourse.bass_utils import run_bass_kernel_spmd

    if "nc" not in _CACHE:
        _CACHE["nc"] = _build_nc()
    nc = _CACHE["nc"]
    in_maps = _host_inputs(x, embed_w, q_w, k_w, v_w, o_w, merge_w)
    res = run_bass_kernel_spmd(nc, in_maps, core_ids=list(range(NCORES)))
    # device row r = 512*half + i holds query s = 512*half + 32*(i%16) + i//16
    r = np.arange(S)
    i = r % 512
    s_of_r = 512 * (r // 512) + 32 * (i % 16) + i // 16
    out = np.zeros((B, S, D), dtype=np.float32)
    for k in range(NCORES):
        out[k // (NCORES // B), s_of_r] += res.results[k]["outp"]
    return out.reshape(B, HH, WW, D)
